# revision 11
# baseline (speedup 1.0000x reference)
"""Trainium2 kernel for BufferRetrievalHungarianMatcher.

Problem: outputs [16,256,2048] f32, targets [16,256,2048] f32.
  cost[b,n,o] = -<outputs[b,n,:], targets[b,o,:]>
  col[b] = Hungarian(cost[b]) (exact min-cost assignment, 256x256)
  return stack([arange(256), col], axis=1) -> [16,2,256] int32

Device side (8 NeuronCores, 2 batches/core): the memory-bound batched
matmul producing the cost slabs. Operands are pre-laid-out on the host so
the contraction dim (2048) lands on SBUF partitions (m-tile-major layout),
avoiding any on-chip transposes; the negation is folded into the host
layout pass. The exact per-sample Hungarian solve (tiny, sequential,
data-dependent) runs on the host on the device-computed cost slabs.
"""

import numpy as np

_NCORES = 8
_B, _N, _M = 16, 256, 2048
_BPC = _B // _NCORES      # batches per core
_MT = _M // 128           # 16 m-tiles of the contraction dim
_NT = _N // 128           # 2 n-tiles (PSUM partition tiles)
_CHUNK = 4                # m-tiles per input DMA (512KB per chunk)

LAST_RESULTS = None       # BassKernelResults of the most recent device run

_nc_cache = None


def _build_nc():
    """Build the SPMD Bass module (one NEFF, run on all 8 cores)."""
    import concourse.mybir as mybir
    from concourse import bacc
    from concourse.tile import TileContext

    f32 = mybir.dt.float32
    nc = bacc.Bacc(
        trn_type="TRN2",
        target_bir_lowering=False,
        debug=False,
        num_devices=_NCORES,
    )
    # Host layout, per batch b and chunk ch (covering m-tiles 4ch..4ch+3):
    #   ab[b, ch, p, loc*256 + n]        = -outputs[2c+b, n, (4ch+loc)*128 + p]
    #   ab[b, ch, p, 1024 + loc*256 + o] =  targets[2c+b, o, (4ch+loc)*128 + p]
    # A and B chunks share one DRAM tensor/tile so each matmul depends on a
    # single input DMA (HW limits sync-wait slots per instruction).
    n_chunks = _MT // _CHUNK
    half = _CHUNK * 256
    ab = nc.dram_tensor(
        "ab", [_BPC, n_chunks, 128, 2 * half], f32, kind="ExternalInput"
    ).ap()
    # One output tensor per batch (separate tensors avoid false WAW deps
    # between the tail DMAs, which would exceed the 1-wait HWDGE limit):
    # cost{b}[p, nt*256 + o] = cost[2c+b, nt*128+p, o]
    costs = [
        nc.dram_tensor(f"cost{b}", [128, _NT * 256], f32, kind="ExternalOutput").ap()
        for b in range(_BPC)
    ]

    with TileContext(nc) as tc:
        with (
            tc.tile_pool(name="inp", bufs=2) as inp,
            tc.tile_pool(name="psum", bufs=2, space="PSUM") as psp,
            tc.tile_pool(name="outp", bufs=2) as outp,
        ):
            for b in range(_BPC):
                ab_sb = []
                for ch in range(n_chunks):
                    t = inp.tile([128, 2 * half], f32, tag=f"ab{ch}", name=f"ab{ch}_{b}")
                    nc.sync.dma_start(t, ab[b, ch])
                    ab_sb.append(t)

                psums = [
                    psp.tile([128, 256], f32, tag=f"c{nt}", name=f"c{nt}_{b}")
                    for nt in range(_NT)
                ]
                for mt in range(_MT):
                    ch, loc = divmod(mt, _CHUNK)
                    rhs = ab_sb[ch][:, half + loc * 256 : half + (loc + 1) * 256]
                    for nt in range(_NT):
                        lo = loc * 256 + nt * 128
                        lhsT = ab_sb[ch][:, lo : lo + 128]
                        nc.tensor.matmul(
                            psums[nt],
                            lhsT,
                            rhs,
                            start=(mt == 0),
                            stop=(mt == _MT - 1),
                        )
                o_t = outp.tile([128, _NT * 256], f32, tag="o", name=f"o_{b}")
                for nt in range(_NT):
                    nc.vector.tensor_copy(
                        o_t[:, nt * 256 : (nt + 1) * 256], psums[nt]
                    )
                nc.sync.dma_start(costs[b], o_t)
    nc.compile()
    return nc


def _get_nc():
    global _nc_cache
    if _nc_cache is None:
        _nc_cache = _build_nc()
    return _nc_cache


def _device_cost(outputs: np.ndarray, targets: np.ndarray) -> np.ndarray:
    """Compute cost[b,n,o] = -outputs[b]@targets[b].T on the 8 NeuronCores."""
    global LAST_RESULTS
    from concourse.bass_utils import run_bass_kernel_spmd

    # [B, N, M] -> m-tile-major transposed layout [B, n_chunks, 128, CHUNK*256]
    n_chunks = _MT // _CHUNK
    half = _CHUNK * 256

    def to_mtile_major(x):
        x = x.reshape(_B, _N, n_chunks, _CHUNK, 128).transpose(0, 2, 4, 3, 1)
        return np.ascontiguousarray(x, dtype=np.float32).reshape(
            _B, n_chunks, 128, half
        )

    ab = np.empty((_B, n_chunks, 128, 2 * half), dtype=np.float32)
    ab[..., :half] = to_mtile_major(outputs)
    np.negative(ab[..., :half], out=ab[..., :half])
    ab[..., half:] = to_mtile_major(targets)

    in_maps = [
        {"ab": ab[c * _BPC : (c + 1) * _BPC]} for c in range(_NCORES)
    ]
    res = run_bass_kernel_spmd(_get_nc(), in_maps, list(range(_NCORES)))
    LAST_RESULTS = res
    cost = np.empty((_B, _N, _N), dtype=np.float32)
    for c in range(_NCORES):
        for b in range(_BPC):
            out = res.results[c][f"cost{b}"]  # [128, NT*256]
            cost[c * _BPC + b] = (
                out.reshape(128, _NT, 256).transpose(1, 0, 2).reshape(_N, _N)
            )
    return cost


def _lap_numpy(cost: np.ndarray) -> np.ndarray:
    """Jonker-Volgenant shortest-augmenting-path LAP (e-maxx form), numpy.

    Fallback when scipy is unavailable. Matches
    scipy.optimize.linear_sum_assignment for square inputs.
    Returns col[row] int32 [n].
    """
    n = cost.shape[0]
    C = np.zeros((n + 1, n + 1), dtype=cost.dtype)
    C[1:, 1:] = cost
    INF = np.inf
    u = np.zeros(n + 1, cost.dtype)
    v = np.zeros(n + 1, cost.dtype)
    p = np.zeros(n + 1, np.int64)
    for i in range(1, n + 1):
        p[0] = i
        j0 = 0
        minv = np.full(n + 1, INF, cost.dtype)
        way = np.zeros(n + 1, np.int64)
        used = np.zeros(n + 1, bool)
        while True:
            used[j0] = True
            i0 = p[j0]
            cur = C[i0] - u[i0] - v
            better = (cur < minv) & ~used
            minv[better] = cur[better]
            way[better] = j0
            masked = np.where(used, INF, minv)
            j1 = int(np.argmin(masked))
            delta = masked[j1]
            add = np.where(used, delta, 0.0).astype(cost.dtype)
            np.add.at(u, p[used], delta)
            v[used] -= delta
            minv[~used] -= delta
            j0 = j1
            if p[j0] == 0:
                break
        while j0 != 0:
            j1 = way[j0]
            p[j0] = p[j1]
            j0 = j1
    col = np.zeros(n, np.int32)
    col[p[1:] - 1] = np.arange(n, dtype=np.int32)
    return col


def _solve_lap(cost: np.ndarray) -> np.ndarray:
    """Per-batch exact assignment: col indices [B, N] int32."""
    try:
        from scipy.optimize import linear_sum_assignment

        return np.stack(
            [
                linear_sum_assignment(cost[b])[1].astype(np.int32)
                for b in range(cost.shape[0])
            ]
        )
    except ImportError:
        return np.stack([_lap_numpy(cost[b]) for b in range(cost.shape[0])])


def kernel(outputs: np.ndarray, targets: np.ndarray) -> np.ndarray:
    outputs = np.asarray(outputs, dtype=np.float32)
    targets = np.asarray(targets, dtype=np.float32)
    cost = _device_cost(outputs, targets)
    col = _solve_lap(cost)
    rows = np.broadcast_to(np.arange(_N, dtype=np.int32), (_B, _N))
    return np.stack([rows, col], axis=1).astype(np.int32)


# revision 17
# speedup vs baseline: 1.0955x; 1.0955x over previous
"""Trainium2 kernel for BufferRetrievalHungarianMatcher.

Problem: outputs [16,256,2048] f32, targets [16,256,2048] f32.
  cost[b,n,o] = -<outputs[b,n,:], targets[b,o,:]>
  col[b] = Hungarian(cost[b]) (exact min-cost assignment, 256x256)
  return stack([arange(256), col], axis=1) -> [16,2,256] int32

Device side (8 NeuronCores, 2 batches/core): the memory-bound batched
matmul producing the cost slabs. Operands are pre-laid-out on the host so
the contraction dim (2048) lands on SBUF partitions (m-tile-major layout),
avoiding any on-chip transposes; the negation is folded into the host
layout pass. The exact per-sample Hungarian solve (tiny, sequential,
data-dependent) runs on the host on the device-computed cost slabs.
"""

import numpy as np

_NCORES = 8
_B, _N, _M = 16, 256, 2048
_BPC = _B // _NCORES      # batches per core
_MT = _M // 128           # 16 m-tiles of the contraction dim
_NT = _N // 128           # 2 n-tiles (PSUM partition tiles)
_CHUNK = 4                # m-tiles per input DMA (512KB per chunk)

LAST_RESULTS = None       # BassKernelResults of the most recent device run

_COMPUTE_DTYPE = "float32"  # "float32" | "float32r" (PE matmul operand dtype)
_nc_cache = {}


def _build_nc(compute_dtype: str = "float32"):
    """Build the SPMD Bass module (one NEFF, run on all 8 cores)."""
    import concourse.mybir as mybir
    from concourse import bacc
    from concourse.tile import TileContext

    f32 = getattr(mybir.dt, compute_dtype)
    nc = bacc.Bacc(
        trn_type="TRN2",
        target_bir_lowering=False,
        debug=False,
        num_devices=_NCORES,
    )
    # Host layout, per batch b and chunk ch (covering m-tiles 4ch..4ch+3):
    #   ab[b, ch, p, loc*256 + n]        = -outputs[2c+b, n, (4ch+loc)*128 + p]
    #   ab[b, ch, p, 1024 + loc*256 + o] =  targets[2c+b, o, (4ch+loc)*128 + p]
    # A and B chunks share one DRAM tensor/tile so each matmul depends on a
    # single input DMA (HW limits sync-wait slots per instruction).
    n_chunks = _MT // _CHUNK
    half = _CHUNK * 256
    ab = nc.dram_tensor(
        "ab", [_BPC, n_chunks, 128, 2 * half], f32, kind="ExternalInput"
    ).ap()
    # One output tensor per batch (separate tensors avoid false WAW deps
    # between the tail DMAs, which would exceed the 1-wait HWDGE limit):
    # cost{b}[p, nt*256 + o] = cost[2c+b, nt*128+p, o]
    of32 = mybir.dt.float32
    costs = [
        nc.dram_tensor(f"cost{b}", [128, _NT * 256], of32, kind="ExternalOutput").ap()
        for b in range(_BPC)
    ]

    with TileContext(nc) as tc:
        with (
            tc.tile_pool(name="inp", bufs=2) as inp,
            tc.tile_pool(name="psum", bufs=2, space="PSUM") as psp,
            tc.tile_pool(name="outp", bufs=2) as outp,
        ):
            for b in range(_BPC):
                ab_sb = []
                for ch in range(n_chunks):
                    t = inp.tile([128, 2 * half], f32, tag=f"ab{ch}", name=f"ab{ch}_{b}")
                    nc.sync.dma_start(t, ab[b, ch])
                    ab_sb.append(t)

                psums = [
                    psp.tile([128, 256], of32, tag=f"c{nt}", name=f"c{nt}_{b}")
                    for nt in range(_NT)
                ]
                for mt in range(_MT):
                    ch, loc = divmod(mt, _CHUNK)
                    rhs = ab_sb[ch][:, half + loc * 256 : half + (loc + 1) * 256]
                    for nt in range(_NT):
                        lo = loc * 256 + nt * 128
                        lhsT = ab_sb[ch][:, lo : lo + 128]
                        nc.tensor.matmul(
                            psums[nt],
                            lhsT,
                            rhs,
                            start=(mt == 0),
                            stop=(mt == _MT - 1),
                        )
                o_t = outp.tile([128, _NT * 256], of32, tag="o", name=f"o_{b}")
                for nt in range(_NT):
                    nc.vector.tensor_copy(
                        o_t[:, nt * 256 : (nt + 1) * 256], psums[nt]
                    )
                nc.sync.dma_start(costs[b], o_t)
    nc.compile()
    return nc


def _get_nc():
    if _COMPUTE_DTYPE not in _nc_cache:
        _nc_cache[_COMPUTE_DTYPE] = _build_nc(_COMPUTE_DTYPE)
    return _nc_cache[_COMPUTE_DTYPE]


def _device_cost(outputs: np.ndarray, targets: np.ndarray) -> np.ndarray:
    """Compute cost[b,n,o] = -outputs[b]@targets[b].T on the 8 NeuronCores."""
    global LAST_RESULTS
    from concourse.bass_utils import run_bass_kernel_spmd

    # [B, N, M] -> m-tile-major transposed layout [B, n_chunks, 128, CHUNK*256]
    n_chunks = _MT // _CHUNK
    half = _CHUNK * 256

    def to_mtile_major(x):
        x = x.reshape(_B, _N, n_chunks, _CHUNK, 128).transpose(0, 2, 4, 3, 1)
        return np.ascontiguousarray(x, dtype=np.float32).reshape(
            _B, n_chunks, 128, half
        )

    ab = np.empty((_B, n_chunks, 128, 2 * half), dtype=np.float32)
    ab[..., :half] = to_mtile_major(outputs)
    np.negative(ab[..., :half], out=ab[..., :half])
    ab[..., half:] = to_mtile_major(targets)

    in_maps = [
        {"ab": ab[c * _BPC : (c + 1) * _BPC]} for c in range(_NCORES)
    ]
    res = run_bass_kernel_spmd(_get_nc(), in_maps, list(range(_NCORES)))
    LAST_RESULTS = res
    cost = np.empty((_B, _N, _N), dtype=np.float32)
    for c in range(_NCORES):
        for b in range(_BPC):
            out = res.results[c][f"cost{b}"]  # [128, NT*256]
            cost[c * _BPC + b] = (
                out.reshape(128, _NT, 256).transpose(1, 0, 2).reshape(_N, _N)
            )
    return cost


def _lap_numpy(cost: np.ndarray) -> np.ndarray:
    """Jonker-Volgenant shortest-augmenting-path LAP (e-maxx form), numpy.

    Fallback when scipy is unavailable. Matches
    scipy.optimize.linear_sum_assignment for square inputs.
    Returns col[row] int32 [n].
    """
    n = cost.shape[0]
    C = np.zeros((n + 1, n + 1), dtype=cost.dtype)
    C[1:, 1:] = cost
    INF = np.inf
    u = np.zeros(n + 1, cost.dtype)
    v = np.zeros(n + 1, cost.dtype)
    p = np.zeros(n + 1, np.int64)
    for i in range(1, n + 1):
        p[0] = i
        j0 = 0
        minv = np.full(n + 1, INF, cost.dtype)
        way = np.zeros(n + 1, np.int64)
        used = np.zeros(n + 1, bool)
        while True:
            used[j0] = True
            i0 = p[j0]
            cur = C[i0] - u[i0] - v
            better = (cur < minv) & ~used
            minv[better] = cur[better]
            way[better] = j0
            masked = np.where(used, INF, minv)
            j1 = int(np.argmin(masked))
            delta = masked[j1]
            add = np.where(used, delta, 0.0).astype(cost.dtype)
            np.add.at(u, p[used], delta)
            v[used] -= delta
            minv[~used] -= delta
            j0 = j1
            if p[j0] == 0:
                break
        while j0 != 0:
            j1 = way[j0]
            p[j0] = p[j1]
            j0 = j1
    col = np.zeros(n, np.int32)
    col[p[1:] - 1] = np.arange(n, dtype=np.int32)
    return col


def _solve_lap(cost: np.ndarray) -> np.ndarray:
    """Per-batch exact assignment: col indices [B, N] int32."""
    try:
        from scipy.optimize import linear_sum_assignment

        return np.stack(
            [
                linear_sum_assignment(cost[b])[1].astype(np.int32)
                for b in range(cost.shape[0])
            ]
        )
    except ImportError:
        return np.stack([_lap_numpy(cost[b]) for b in range(cost.shape[0])])


def kernel(outputs: np.ndarray, targets: np.ndarray) -> np.ndarray:
    outputs = np.asarray(outputs, dtype=np.float32)
    targets = np.asarray(targets, dtype=np.float32)
    cost = _device_cost(outputs, targets)
    col = _solve_lap(cost)
    rows = np.broadcast_to(np.arange(_N, dtype=np.int32), (_B, _N))
    return np.stack([rows, col], axis=1).astype(np.int32)


# revision 19
# speedup vs baseline: 1.2517x; 1.1426x over previous
"""Trainium2 kernel for BufferRetrievalHungarianMatcher.

Problem: outputs [16,256,2048] f32, targets [16,256,2048] f32.
  cost[b,n,o] = -<outputs[b,n,:], targets[b,o,:]>
  col[b] = Hungarian(cost[b]) (exact min-cost assignment, 256x256)
  return stack([arange(256), col], axis=1) -> [16,2,256] int32

Device side (8 NeuronCores, 2 batches/core): the memory-bound batched
matmul producing the cost slabs. Operands are pre-laid-out on the host so
the contraction dim (2048) lands on SBUF partitions (m-tile-major layout),
avoiding any on-chip transposes; the negation is folded into the host
layout pass. The exact per-sample Hungarian solve (tiny, sequential,
data-dependent) runs on the host on the device-computed cost slabs.
"""

import numpy as np

_NCORES = 8
_B, _N, _M = 16, 256, 2048
_BPC = _B // _NCORES      # batches per core
_MT = _M // 128           # 16 m-tiles of the contraction dim
_NT = _N // 128           # 2 n-tiles (PSUM partition tiles)
_CHUNK = 4                # m-tiles per input DMA (512KB per chunk)

LAST_RESULTS = None       # BassKernelResults of the most recent device run

_COMPUTE_DTYPE = "float32"  # "float32" | "float32r" (PE matmul operand dtype)
_nc_cache = {}


def _build_nc(compute_dtype: str = "float32"):
    """Build the SPMD Bass module (one NEFF, run on all 8 cores)."""
    import concourse.mybir as mybir
    from concourse import bacc
    from concourse.tile import TileContext

    f32 = getattr(mybir.dt, compute_dtype)
    nc = bacc.Bacc(
        trn_type="TRN2",
        target_bir_lowering=False,
        debug=False,
        num_devices=_NCORES,
    )
    # Host layout, per batch b and chunk ch (covering m-tiles 4ch..4ch+3):
    #   ab[b, ch, p, loc*256 + n]        = -outputs[2c+b, n, (4ch+loc)*128 + p]
    #   ab[b, ch, p, 1024 + loc*256 + o] =  targets[2c+b, o, (4ch+loc)*128 + p]
    # A and B chunks share one DRAM tensor/tile so each matmul depends on a
    # single input DMA (HW limits sync-wait slots per instruction).
    n_chunks = _MT // _CHUNK
    half = _CHUNK * 256
    ab = nc.dram_tensor(
        "ab", [_BPC, n_chunks, 128, 2 * half], f32, kind="ExternalInput"
    ).ap()
    # One output tensor per batch (separate tensors avoid false WAW deps
    # between the tail DMAs, which would exceed the 1-wait HWDGE limit):
    # cost{b}[p, nt*256 + o] = cost[2c+b, nt*128+p, o]
    of32 = mybir.dt.float32
    costs = [
        nc.dram_tensor(f"cost{b}", [128, _NT * 256], of32, kind="ExternalOutput").ap()
        for b in range(_BPC)
    ]

    with TileContext(nc) as tc:
        with (
            tc.tile_pool(name="inp", bufs=2) as inp,
            tc.tile_pool(name="psum", bufs=2, space="PSUM") as psp,
            tc.tile_pool(name="outp", bufs=2) as outp,
        ):
            # Issue every input DMA up front on the SP (sync) HWDGE queue so
            # the input stream is never stalled behind an output DMA's wait
            # (the SP sequencer issues strictly in program order). Output
            # DMAs go on the Scalar-engine HWDGE queue instead.
            ab_sb_all = []
            for b in range(_BPC):
                ab_sb = []
                for ch in range(n_chunks):
                    t = inp.tile([128, 2 * half], f32, tag=f"ab{ch}", name=f"ab{ch}_{b}")
                    nc.sync.dma_start(t, ab[b, ch])
                    ab_sb.append(t)
                ab_sb_all.append(ab_sb)

            for b in range(_BPC):
                ab_sb = ab_sb_all[b]
                psums = [
                    psp.tile([128, 256], of32, tag=f"c{nt}", name=f"c{nt}_{b}")
                    for nt in range(_NT)
                ]
                for mt in range(_MT):
                    ch, loc = divmod(mt, _CHUNK)
                    rhs = ab_sb[ch][:, half + loc * 256 : half + (loc + 1) * 256]
                    for nt in range(_NT):
                        lo = loc * 256 + nt * 128
                        lhsT = ab_sb[ch][:, lo : lo + 128]
                        nc.tensor.matmul(
                            psums[nt],
                            lhsT,
                            rhs,
                            start=(mt == 0),
                            stop=(mt == _MT - 1),
                        )
                o_t = outp.tile([128, _NT * 256], of32, tag="o", name=f"o_{b}")
                for nt in range(_NT):
                    nc.vector.tensor_copy(
                        o_t[:, nt * 256 : (nt + 1) * 256], psums[nt]
                    )
                nc.scalar.dma_start(costs[b], o_t)
    nc.compile()
    return nc


def _get_nc():
    if _COMPUTE_DTYPE not in _nc_cache:
        _nc_cache[_COMPUTE_DTYPE] = _build_nc(_COMPUTE_DTYPE)
    return _nc_cache[_COMPUTE_DTYPE]


def _device_cost(outputs: np.ndarray, targets: np.ndarray) -> np.ndarray:
    """Compute cost[b,n,o] = -outputs[b]@targets[b].T on the 8 NeuronCores."""
    global LAST_RESULTS
    from concourse.bass_utils import run_bass_kernel_spmd

    # [B, N, M] -> m-tile-major transposed layout [B, n_chunks, 128, CHUNK*256]
    n_chunks = _MT // _CHUNK
    half = _CHUNK * 256

    def to_mtile_major(x):
        x = x.reshape(_B, _N, n_chunks, _CHUNK, 128).transpose(0, 2, 4, 3, 1)
        return np.ascontiguousarray(x, dtype=np.float32).reshape(
            _B, n_chunks, 128, half
        )

    ab = np.empty((_B, n_chunks, 128, 2 * half), dtype=np.float32)
    ab[..., :half] = to_mtile_major(outputs)
    np.negative(ab[..., :half], out=ab[..., :half])
    ab[..., half:] = to_mtile_major(targets)

    in_maps = [
        {"ab": ab[c * _BPC : (c + 1) * _BPC]} for c in range(_NCORES)
    ]
    res = run_bass_kernel_spmd(_get_nc(), in_maps, list(range(_NCORES)))
    LAST_RESULTS = res
    cost = np.empty((_B, _N, _N), dtype=np.float32)
    for c in range(_NCORES):
        for b in range(_BPC):
            out = res.results[c][f"cost{b}"]  # [128, NT*256]
            cost[c * _BPC + b] = (
                out.reshape(128, _NT, 256).transpose(1, 0, 2).reshape(_N, _N)
            )
    return cost


def _lap_numpy(cost: np.ndarray) -> np.ndarray:
    """Jonker-Volgenant shortest-augmenting-path LAP (e-maxx form), numpy.

    Fallback when scipy is unavailable. Matches
    scipy.optimize.linear_sum_assignment for square inputs.
    Returns col[row] int32 [n].
    """
    n = cost.shape[0]
    C = np.zeros((n + 1, n + 1), dtype=cost.dtype)
    C[1:, 1:] = cost
    INF = np.inf
    u = np.zeros(n + 1, cost.dtype)
    v = np.zeros(n + 1, cost.dtype)
    p = np.zeros(n + 1, np.int64)
    for i in range(1, n + 1):
        p[0] = i
        j0 = 0
        minv = np.full(n + 1, INF, cost.dtype)
        way = np.zeros(n + 1, np.int64)
        used = np.zeros(n + 1, bool)
        while True:
            used[j0] = True
            i0 = p[j0]
            cur = C[i0] - u[i0] - v
            better = (cur < minv) & ~used
            minv[better] = cur[better]
            way[better] = j0
            masked = np.where(used, INF, minv)
            j1 = int(np.argmin(masked))
            delta = masked[j1]
            add = np.where(used, delta, 0.0).astype(cost.dtype)
            np.add.at(u, p[used], delta)
            v[used] -= delta
            minv[~used] -= delta
            j0 = j1
            if p[j0] == 0:
                break
        while j0 != 0:
            j1 = way[j0]
            p[j0] = p[j1]
            j0 = j1
    col = np.zeros(n, np.int32)
    col[p[1:] - 1] = np.arange(n, dtype=np.int32)
    return col


def _solve_lap(cost: np.ndarray) -> np.ndarray:
    """Per-batch exact assignment: col indices [B, N] int32."""
    try:
        from scipy.optimize import linear_sum_assignment

        return np.stack(
            [
                linear_sum_assignment(cost[b])[1].astype(np.int32)
                for b in range(cost.shape[0])
            ]
        )
    except ImportError:
        return np.stack([_lap_numpy(cost[b]) for b in range(cost.shape[0])])


def kernel(outputs: np.ndarray, targets: np.ndarray) -> np.ndarray:
    outputs = np.asarray(outputs, dtype=np.float32)
    targets = np.asarray(targets, dtype=np.float32)
    cost = _device_cost(outputs, targets)
    col = _solve_lap(cost)
    rows = np.broadcast_to(np.arange(_N, dtype=np.int32), (_B, _N))
    return np.stack([rows, col], axis=1).astype(np.int32)
